# revision 1
# baseline (speedup 1.0000x reference)
"""Trainium2 Bass kernel for nn_BridgeModule (vision->text cross-attention + FFN).

Strategy: data-parallel over batch (B=8, one batch element per NeuronCore).
Dataflow is channel-major (features on SBUF partitions, tokens on the free
dim), so every matmul consumes weights in their natural [Cin, Cout] layout
and per-channel biases fuse into PSUM eviction as per-partition scalars.
Matmuls run in bf16 with fp32 PSUM accumulation.

Layout tricks:
  - head dim DK=288 zero-padded to 384 (3x128) so per-head contraction
    chunks are partition-aligned
  - vision tokens SV=257 zero-padded to 384; pad keys are masked by zeroing
    their exp() rows before the attention-value matmul
  - softmax runs without max-subtraction (scores are O(1) by construction)
  - LayerNorm stats (sums over channels = over partitions) via ones-matmuls
  - Q, x (post-attention residual), and the FFN hidden h spill to DRAM

All host-side preprocessing (transposes, padding, bf16 casts, SBUF-image
tiling) happens in numpy inside kernel(); the device sees ready-to-DMA
layouts.
"""

import numpy as np
import ml_dtypes

import concourse.bass as bass
import concourse.tile as tile
import concourse.mybir as mybir
from concourse import bacc
from concourse.bass_utils import run_bass_kernel_spmd

# ---------------------------------------------------------------- constants
B, SV, SQ = 8, 257, 2048
DV, DM, H = 1024, 2304, 8
DK = DM // H            # 288
DKP = 384               # padded head dim (3 x 128)
DQP = H * DKP           # 3072
DF = 4 * DM             # 9216
SVP = 384               # padded vision tokens
EPS = 1e-5
P = 128
SCALE = 1.0 / float(np.sqrt(np.float32(DK)))

KO_DM = DM // P         # 18
KO_QP = DQP // P        # 24
KO_DV = DV // P         # 8
KO_DF = DF // P         # 72
HC = DKP // P           # 3 contraction chunks per head
ST = SVP // P           # 3 vision-token partition tiles
NB = 2                  # attention token blocks
NBS = SQ // NB          # 1024
NT = SQ // 512          # matmul free-dim tiles of 512

BF = mybir.dt.bfloat16
F32 = mybir.dt.float32
bf16 = ml_dtypes.bfloat16

AF = mybir.ActivationFunctionType
OP = mybir.AluOpType

_NC_CACHE = {}


def _dq(nc, i):
    """Alternate bulk DMAs between the two HW DGE queues (SP / ACT)."""
    return nc.sync if i % 2 == 0 else nc.scalar


def _pbcast(ap2d, p=P):
    """[1, ...] AP -> [p, ...] AP with partition stride 0 (for DMA broadcast)."""
    aplist = [list(x) for x in ap2d.ap]
    return bass.AP(tensor=ap2d.tensor, offset=ap2d.offset,
                   ap=[[0, p]] + aplist[1:])


def _build_nc():
    nc = bacc.Bacc(target_bir_lowering=False)
    with tile.TileContext(nc) as tc:
        _emit(nc, tc)
    nc.compile()
    return nc


def _emit(nc, tc):
    with tc.tile_pool(name="dram", bufs=1, space="DRAM") as dram:
        # ---------------- external I/O (SBUF-image layouts, host-prepped)
        def ein(name, shape, dtype):
            return dram.tile(list(shape), dtype, kind="ExternalInput",
                             name=name, uniquify=False)

        te = ein("te", [P, KO_DM, SQ], BF)
        vf = ein("vf", [P, KO_DV, SVP], BF)
        vp_wt = ein("vp_wt", [KO_DM, P, KO_DV, P], BF)
        wq_t = ein("wq_t", [KO_QP, P, KO_DM, P], BF)
        wk_t = ein("wk_t", [KO_QP, P, KO_DM, P], BF)
        wv_r = ein("wv_r", [DQP // 512, P, KO_DM, 512], BF)
        wo_t = ein("wo_t", [KO_DM, P, KO_QP, P], BF)
        f1_t = ein("f1_t", [KO_DF, P, KO_DM, P], BF)
        f2_t = ein("f2_t", [KO_DM, P, KO_DF, P], BF)
        vp_bt = ein("vp_bt", [P, KO_DM], F32)
        wqb_t = ein("wqb_t", [P, KO_QP], F32)
        wkb_t = ein("wkb_t", [P, KO_QP], F32)
        wvb = ein("wvb", [1, DQP], F32)
        wob_t = ein("wob_t", [P, KO_DM], F32)
        f1b_t = ein("f1b_t", [P, KO_DF], F32)
        f2b_t = ein("f2b_t", [P, KO_DM], F32)
        ln1w_t = ein("ln1w_t", [P, KO_DM], F32)
        ln1b_t = ein("ln1b_t", [P, KO_DM], F32)
        ln2w_t = ein("ln2w_t", [P, KO_DM], F32)
        ln2b_t = ein("ln2b_t", [P, KO_DM], F32)
        out = dram.tile([P, KO_DM, SQ], F32, kind="ExternalOutput",
                        name="out", uniquify=False)
        x_out = dram.tile([P, KO_DM, SQ], F32, kind="ExternalOutput",
                          name="x_out", uniquify=False)

        # DRAM scratch
        q_dram = dram.tile([P, KO_QP, SQ], BF, name="q_dram")
        h_dram = dram.tile([P, KO_DF, SQ], BF, name="h_dram")

        with tc.tile_pool(name="consts", bufs=1) as consts, \
             tc.tile_pool(name="psum", bufs=4, space="PSUM") as psum, \
             tc.tile_pool(name="psum1", bufs=2, space="PSUM") as psum1:

            ones_bf = consts.tile([P, 1], BF)
            nc.vector.memset(ones_bf[:], 1.0)
            ones_f = consts.tile([P, 1], F32)
            nc.vector.memset(ones_f[:], 1.0)

            def cload(src, shape):
                t = consts.tile(list(shape), F32, tag=f"c_{src.name}")
                nc.sync.dma_start(t[:], src[:])
                return t

            vp_b = cload(vp_bt, [P, KO_DM])
            wq_b = cload(wqb_t, [P, KO_QP])
            wk_b = cload(wkb_t, [P, KO_QP])
            wo_b = cload(wob_t, [P, KO_DM])
            f1_b = cload(f1b_t, [P, KO_DF])
            f2_b = cload(f2b_t, [P, KO_DM])
            ln1w = cload(ln1w_t, [P, KO_DM])
            ln1b = cload(ln1b_t, [P, KO_DM])
            ln2w = cload(ln2w_t, [P, KO_DM])
            ln2b = cload(ln2b_t, [P, KO_DM])
            x2sums = dram.tile([1, SQ], F32, name="x2sums")
            x2sumsq = dram.tile([1, SQ], F32, name="x2sumsq")

            import os
            kph = int(os.environ.get("KPH", "7"))
            with tc.tile_pool(name="kvpool", bufs=1) as kvpool:
                kcm = kvpool.tile([P, KO_QP, SVP], BF)   # keys, channel-major
                v_tm = kvpool.tile([P, ST, DQP], BF)     # values, token-major
                _vision_kv(nc, tc, psum, vf, vp_wt, wk_t, wv_r,
                           vp_b, wk_b, wvb, kcm, v_tm)

                with tc.tile_pool(name="ntpool", bufs=1) as ntpool:
                    nt = ntpool.tile([P, KO_DM, SQ], BF)
                    if kph >= 2:
                        _ln_cm(nc, tc, psum1, ones_bf, nt, te, dram,
                               ln1w, ln1b, "ln1")
                    if kph >= 3:
                        _q_proj(nc, tc, psum, nt, wq_t, wq_b, q_dram)

                rec_dram = dram.tile([1, NB * H * NBS], F32, name="rec_dram")
                if kph >= 4:
                    _attention(nc, tc, psum, psum1, ones_bf, ones_f, kcm, v_tm,
                               q_dram, wo_t, wo_b, te, x_out, x2sums, x2sumsq,
                               rec_dram)

            with tc.tile_pool(name="nxpool", bufs=1) as nxpool:
                nx = nxpool.tile([P, KO_DM, SQ], BF)
                if kph >= 5:
                    _ln_precomputed(nc, tc, nx, x_out, x2sums, x2sumsq,
                                    ln2w, ln2b, "ln2", dram)
                if kph >= 6:
                    _ffn1(nc, tc, psum, nx, f1_t, f1_b, h_dram)

            if kph >= 7:
                _ffn2(nc, tc, psum, h_dram, f2_t, f2_b, out)


def _vision_kv(nc, tc, psum, vf, vp_wt, wk_t, wv_r, vp_b, wk_b, wvb,
               kcm, v_tm):
    """pv = vp_w.T @ vf + vp_b; keys kcm = wk.T @ pv + wk_b (channel-major);
    values v_tm = pv.T @ wv + wv_b (token-major)."""
    with tc.tile_pool(name="vision", bufs=1) as vision, \
         tc.tile_pool(name="vwork", bufs=3) as vwork:
        wv_bb = vision.tile([P, DQP], F32)
        nc.sync.dma_start(wv_bb[:], _pbcast(wvb[:]))
        vf_sb = vision.tile([P, KO_DV, SVP], BF)
        nc.sync.dma_start(vf_sb[:], vf[:])
        pv = vision.tile([P, KO_DM, SVP], BF)
        for m in range(KO_DM):
            w_sl = vwork.tile([P, KO_DV, P], BF, tag="vp_sl")
            nc.sync.dma_start(w_sl[:], vp_wt[m])
            ps = psum.tile([P, 512], F32, tag="ps_a")
            for k in range(KO_DV):
                nc.tensor.matmul(ps[:, :SVP], w_sl[:, k], vf_sb[:, k],
                                 start=(k == 0), stop=(k == KO_DV - 1))
            nc.scalar.activation(pv[:, m], ps[:, :SVP], AF.Identity,
                                 bias=vp_b[:, m:m + 1])

        for m in range(KO_QP):
            w_sl = vwork.tile([P, KO_DM, P], BF, tag="wk_sl")
            _dq(nc, m).dma_start(w_sl[:], wk_t[m])
            ps = psum.tile([P, 512], F32, tag="ps_a")
            for k in range(KO_DM):
                nc.tensor.matmul(ps[:, :SVP], w_sl[:, k], pv[:, k],
                                 start=(k == 0), stop=(k == KO_DM - 1))
            nc.scalar.activation(kcm[:, m], ps[:, :SVP], AF.Identity,
                                 bias=wk_b[:, m:m + 1])

        for n in range(DQP // 512):
            w_sl = vwork.tile([P, KO_DM, 512], BF, tag="wv_sl", bufs=2)
            _dq(nc, n).dma_start(w_sl[:], wv_r[n])
            for st in range(ST):
                ps = psum.tile([P, 512], F32, tag="ps_a")
                for k in range(KO_DM):
                    nc.tensor.matmul(ps[:], pv[:, k, st * P:(st + 1) * P],
                                     w_sl[:, k],
                                     start=(k == 0), stop=(k == KO_DM - 1))
                nc.vector.scalar_tensor_tensor(
                    v_tm[:, st, n * 512:(n + 1) * 512], ps[:], 1.0,
                    wv_bb[:, n * 512:(n + 1) * 512], OP.mult, OP.add)


def _ln_cm(nc, tc, psum1, ones_bf, out_bf, src_dram, dram, w, b, nm):
    """LayerNorm over channels (partition dim), channel-major. Loads src from
    DRAM (fp32 [P, KO_DM, SQ]), writes normalized bf16 into out_bf in place."""
    with tc.tile_pool(name=nm, bufs=1) as pool, \
         tc.tile_pool(name=nm + "w", bufs=2) as work:
        for m in range(KO_DM):
            _dq(nc, m).dma_start(out_bf[:, m], src_dram[:, m])
        sums = pool.tile([1, SQ], F32)
        sumsq = pool.tile([1, SQ], F32)
        for n in range(NT):
            nsl = slice(n * 512, (n + 1) * 512)
            ps_s = psum1.tile([1, 512], F32, tag="ps_sum")
            ps_q = psum1.tile([1, 512], F32, tag="ps_sq")
            for m in range(KO_DM):
                nc.tensor.matmul(ps_s[:], ones_bf[:], out_bf[:, m, nsl],
                                 start=(m == 0), stop=(m == KO_DM - 1))
            for m in range(KO_DM):
                sq = work.tile([P, 512], BF, tag="sq")
                nc.vector.tensor_mul(sq[:], out_bf[:, m, nsl], out_bf[:, m, nsl])
                nc.tensor.matmul(ps_q[:], ones_bf[:], sq[:],
                                 start=(m == 0), stop=(m == KO_DM - 1))
            nc.vector.tensor_copy(sums[:, nsl], ps_s[:])
            nc.vector.tensor_copy(sumsq[:, nsl], ps_q[:])
        m_b, r_b = _ln_finalize(nc, pool, sums, sumsq, dram, nm)
        _ln_apply(nc, work, out_bf, out_bf, m_b, r_b, w, b)


def _ln_finalize(nc, pool, sums, sumsq, dram, nm):
    """sums/sumsq [1, SQ] (modified in place) -> broadcast mean/rstd [P, SQ]."""
    tmp = pool.tile([1, SQ], F32, tag="ln_fin_tmp")
    nc.vector.tensor_scalar_mul(sums[:], sums[:], 1.0 / DM)      # mean
    nc.vector.tensor_scalar_mul(sumsq[:], sumsq[:], 1.0 / DM)
    nc.vector.scalar_tensor_tensor(tmp[:], sums[:], 1.0, sums[:],
                                   OP.mult, OP.mult)             # mean^2
    nc.vector.tensor_sub(sumsq[:], sumsq[:], tmp[:])             # var
    eps_t = pool.tile([1, 1], F32, tag="ln_eps")
    nc.vector.memset(eps_t[:], EPS)
    nc.scalar.activation(tmp[:], sumsq[:], AF.Sqrt, bias=eps_t[:])  # std
    nc.vector.reciprocal(sumsq[:], tmp[:])                       # rstd
    # SBUF->SBUF partition broadcast is illegal; bounce through DRAM.
    m_dram = dram.tile([1, SQ], F32, name=nm + "_m_dram")
    nc.sync.dma_start(m_dram[:], sums[:])
    r_dram = dram.tile([1, SQ], F32, name=nm + "_r_dram")
    nc.sync.dma_start(r_dram[:], sumsq[:])
    m_b = pool.tile([P, SQ], F32, tag="ln_m_b")
    nc.sync.dma_start(m_b[:], _pbcast(m_dram[:]))
    r_b = pool.tile([P, SQ], F32, tag="ln_r_b")
    nc.sync.dma_start(r_b[:], _pbcast(r_dram[:]))
    return m_b, r_b


def _ln_apply(nc, work, out_bf, src, m_b, r_b, w, b):
    for m in range(KO_DM):
        tmp = work.tile([P, SQ], BF, tag="ln_tmp")
        nc.vector.tensor_sub(tmp[:], src[:, m], m_b[:])
        nc.vector.scalar_tensor_tensor(out_bf[:, m], tmp[:], w[:, m:m + 1],
                                       r_b[:], OP.mult, OP.mult)
        nc.vector.tensor_scalar_add(out_bf[:, m], out_bf[:, m], b[:, m:m + 1])


def _ln_precomputed(nc, tc, nx, x_dram, sums_dram, sumsq_dram, w, b, nm, dram=None):
    """LN whose sums/sumsq were accumulated earlier (in DRAM); reads x from DRAM."""
    with tc.tile_pool(name=nm, bufs=1) as pool, \
         tc.tile_pool(name=nm + "w", bufs=2) as work:
        sums = pool.tile([1, SQ], F32, tag="ln_sums")
        nc.sync.dma_start(sums[:], sums_dram[:])
        sumsq = pool.tile([1, SQ], F32, tag="ln_sumsq")
        nc.sync.dma_start(sumsq[:], sumsq_dram[:])
        m_b, r_b = _ln_finalize(nc, pool, sums, sumsq, dram, nm)
        for m in range(KO_DM):
            x_sl = work.tile([P, SQ], F32, tag="x_sl")
            _dq(nc, m).dma_start(x_sl[:], x_dram[:, m])
            tmp = work.tile([P, SQ], F32, tag="nx_tmp")
            nc.vector.tensor_sub(tmp[:], x_sl[:], m_b[:])
            nc.vector.scalar_tensor_tensor(nx[:, m], tmp[:], w[:, m:m + 1],
                                           r_b[:], OP.mult, OP.mult)
            nc.vector.tensor_scalar_add(nx[:, m], nx[:, m], b[:, m:m + 1])


def _q_proj(nc, tc, psum, nt, wq_t, wq_b, q_dram):
    """Q = (wq_pad.T @ nt)*SCALE + wq_b*SCALE -> DRAM (bias pre-scaled)."""
    with tc.tile_pool(name="qwork", bufs=3) as qwork:
        for m in range(KO_QP):
            w_sl = qwork.tile([P, KO_DM, P], BF, tag="wq_sl")
            _dq(nc, m).dma_start(w_sl[:], wq_t[m])
            for n in range(NT):
                nsl = slice(n * 512, (n + 1) * 512)
                ps = psum.tile([P, 512], F32, tag="ps_a")
                for k in range(KO_DM):
                    nc.tensor.matmul(ps[:], w_sl[:, k], nt[:, k, nsl],
                                     start=(k == 0), stop=(k == KO_DM - 1))
                q_sb = qwork.tile([P, 512], BF, tag="q_sb")
                nc.scalar.activation(q_sb[:], ps[:], AF.Identity,
                                     bias=wq_b[:, m:m + 1], scale=SCALE)
                nc.sync.dma_start(q_dram[:, m, nsl], q_sb[:])


def _attention(nc, tc, psum, psum1, ones_bf, ones_f, kcm, v_tm, q_dram,
               wo_t, wo_b, te, x_out, x2sums, x2sumsq, rec_dram):
    """Per token block (NBS=1024): scoresT, exp (no max-sub, pad masked),
    unnormalized ctx, per-head normalization, O projection + residual, LN2
    stats. x -> x_out (fp32, external); final residual happens on host."""
    with tc.tile_pool(name="attn", bufs=1) as attn, \
         tc.tile_pool(name="awork", bufs=2) as awork:
        for nb in range(NB):
            bsl = slice(nb * NBS, (nb + 1) * NBS)
            q_blk = attn.tile([P, KO_QP, NBS], BF, tag="q_blk")
            _dq(nc, nb).dma_start(q_blk[:], q_dram[:, :, bsl])
            ctx_blk = attn.tile([P, KO_QP, NBS], BF, tag="ctx_blk")
            for h in range(H):
                expT = awork.tile([P, ST, NBS], BF, tag="expT")
                nc.vector.memset(expT[:, ST - 1], 0.0)
                rec = awork.tile([1, NBS], F32, tag="rec")
                for n2 in range(NBS // 512):
                    n2sl = slice(n2 * 512, (n2 + 1) * 512)
                    ps_sum = psum1.tile([1, 512], F32, tag="ps_sum")
                    for st in range(ST):
                        ps_s = psum.tile([P, 512], F32, tag="ps_a")
                        ssl = slice(st * P, (st + 1) * P)
                        for kc in range(HC):
                            nc.tensor.matmul(ps_s[:], kcm[:, HC * h + kc, ssl],
                                             q_blk[:, HC * h + kc, n2sl],
                                             start=(kc == 0), stop=(kc == HC - 1))
                        if st < ST - 1:
                            nc.scalar.activation(expT[:, st, n2sl], ps_s[:], AF.Exp)
                        else:
                            # only vision token 256 is real in the last s-tile
                            nc.scalar.activation(expT[0:1, st, n2sl],
                                                 ps_s[0:1], AF.Exp)
                        nc.tensor.matmul(ps_sum[:], ones_bf[:], expT[:, st, n2sl],
                                         start=(st == 0), stop=(st == ST - 1))
                    nc.vector.reciprocal(rec[:, n2sl], ps_sum[:])
                roff = (nb * H + h) * NBS
                nc.sync.dma_start(rec_dram[:, roff:roff + NBS], rec[:])
                rec_b = awork.tile([P, NBS], F32, tag="rec_b")
                nc.sync.dma_start(rec_b[:], _pbcast(rec_dram[:, roff:roff + NBS]))
                for st in range(ST):
                    nc.vector.tensor_mul(expT[:, st], expT[:, st], rec_b[:])
                for dt3 in range(HC):
                    dsl = slice((HC * h + dt3) * P, (HC * h + dt3 + 1) * P)
                    for n2 in range(NBS // 512):
                        n2sl = slice(n2 * 512, (n2 + 1) * 512)
                        ps_c = psum.tile([P, 512], F32, tag="ps_a")
                        for st in range(ST):
                            nc.tensor.matmul(ps_c[:], v_tm[:, st, dsl],
                                             expT[:, st, n2sl],
                                             start=(st == 0), stop=(st == ST - 1))
                        nc.vector.tensor_copy(ctx_blk[:, HC * h + dt3, n2sl],
                                              ps_c[:])

            # O projection + residual -> x_out (fp32); LN2 stats inline
            # via fp32 ones-matmuls on the transient x_t tiles.
            n_n2 = NBS // 512
            ps_ss = [psum1.tile([1, 512], F32, tag="ps_sum", name=f"ps_ss{_n}")
                     for _n in range(n_n2)]
            ps_qs = [psum1.tile([1, 512], F32, tag="ps_sq", name=f"ps_qs{_n}")
                     for _n in range(n_n2)]
            for m in range(KO_DM):
                w_sl = awork.tile([P, KO_QP, P], BF, tag="wo_sl")
                _dq(nc, m).dma_start(w_sl[:], wo_t[m])
                te_sl = awork.tile([P, NBS], BF, tag="te_res")
                _dq(nc, m + 1).dma_start(te_sl[:], te[:, m, bsl])
                x_t = awork.tile([P, NBS], F32, tag="x_t")
                sq_t = awork.tile([P, NBS], F32, tag="sq_t")
                for n2 in range(n_n2):
                    n2sl = slice(n2 * 512, (n2 + 1) * 512)
                    ps = psum.tile([P, 512], F32, tag="ps_a")
                    for k in range(KO_QP):
                        nc.tensor.matmul(ps[:], w_sl[:, k], ctx_blk[:, k, n2sl],
                                         start=(k == 0), stop=(k == KO_QP - 1))
                    nc.vector.scalar_tensor_tensor(x_t[:, n2sl], ps[:],
                                                   wo_b[:, m:m + 1],
                                                   te_sl[:, n2sl], OP.add, OP.add)
                    nc.tensor.matmul(ps_ss[n2][:], ones_f[:], x_t[:, n2sl],
                                     start=(m == 0), stop=(m == KO_DM - 1))
                    nc.vector.tensor_mul(sq_t[:, n2sl], x_t[:, n2sl],
                                         x_t[:, n2sl])
                    nc.tensor.matmul(ps_qs[n2][:], ones_f[:], sq_t[:, n2sl],
                                     start=(m == 0), stop=(m == KO_DM - 1))
                _dq(nc, m).dma_start(x_out[:, m, bsl], x_t[:])
            for n2 in range(n_n2):
                n2sl = slice(nb * NBS + n2 * 512, nb * NBS + (n2 + 1) * 512)
                s_sb = awork.tile([1, 512], F32, tag="s_sb")
                nc.vector.tensor_copy(s_sb[:], ps_ss[n2][:])
                nc.sync.dma_start(x2sums[:, n2sl], s_sb[:])
                q_sb = awork.tile([1, 512], F32, tag="qs_sb")
                nc.vector.tensor_copy(q_sb[:], ps_qs[n2][:])
                nc.sync.dma_start(x2sumsq[:, n2sl], q_sb[:])


def _ffn1(nc, tc, psum, nx, f1_t, f1_b, h_dram):
    """h = gelu(f1.T @ nx + f1_b) -> DRAM bf16."""
    with tc.tile_pool(name="f1work", bufs=3) as f1work:
        for m in range(KO_DF):
            w_sl = f1work.tile([P, KO_DM, P], BF, tag="f1_sl")
            _dq(nc, m).dma_start(w_sl[:], f1_t[m])
            for n in range(NT):
                nsl = slice(n * 512, (n + 1) * 512)
                ps = psum.tile([P, 512], F32, tag="ps_a")
                for k in range(KO_DM):
                    nc.tensor.matmul(ps[:], w_sl[:, k], nx[:, k, nsl],
                                     start=(k == 0), stop=(k == KO_DM - 1))
                h_sb = f1work.tile([P, 512], BF, tag="h_sb")
                nc.scalar.activation(h_sb[:], ps[:], AF.Gelu,
                                     bias=f1_b[:, m:m + 1])
                nc.sync.dma_start(h_dram[:, m, nsl], h_sb[:])


def _ffn2(nc, tc, psum, h_dram, f2_t, f2_b, out):
    """out = f2.T @ h + f2_b (residual added on host), 1024-token blocks."""
    with tc.tile_pool(name="f2blk", bufs=1) as f2blk, \
         tc.tile_pool(name="f2work", bufs=2) as f2work:
        for nb in range(2):
            nsl = slice(nb * 1024, (nb + 1) * 1024)
            h_blk = f2blk.tile([P, KO_DF, 1024], BF, tag="h_blk")
            _dq(nc, nb).dma_start(h_blk[:], h_dram[:, :, nsl])
            for m in range(KO_DM):
                w_sl = f2work.tile([P, KO_DF, P], BF, tag="f2_sl")
                _dq(nc, m).dma_start(w_sl[:], f2_t[m])
                o_sb = f2work.tile([P, 1024], F32, tag="o_sb")
                for n2 in range(2):
                    n2sl = slice(n2 * 512, (n2 + 1) * 512)
                    ps = psum.tile([P, 512], F32, tag="ps_a")
                    for k in range(KO_DF):
                        nc.tensor.matmul(ps[:], w_sl[:, k], h_blk[:, k, n2sl],
                                         start=(k == 0), stop=(k == KO_DF - 1))
                    nc.scalar.activation(o_sb[:, n2sl], ps[:], AF.Identity,
                                         bias=f2_b[:, m:m + 1])
                _dq(nc, m + 1).dma_start(out[:, m, nb * 1024:(nb + 1) * 1024],
                                         o_sb[:])


# ------------------------------------------------------------- host wrappers

def _tile_w(w, ko, mo):
    """[K, M] weight -> [mo, 128, ko, mi] SBUF-image bf16 tiles."""
    K, M = w.shape
    mi = M // mo
    r = w.reshape(ko, P, mo, mi).transpose(2, 1, 0, 3)
    return np.ascontiguousarray(r.astype(bf16))


def _col_pad_heads(w):
    """[*, 2304] -> [*, 3072] zero-padding each head's 288 cols to 384."""
    r = np.zeros(w.shape[:-1] + (DQP,), np.float32)
    r.reshape(w.shape[:-1] + (H, DKP))[..., :DK] = \
        w.reshape(w.shape[:-1] + (H, DK))
    return r


def _row_pad_heads(w):
    """[2304, *] -> [3072, *] zero-padding each head's 288 rows to 384."""
    r = np.zeros((DQP,) + w.shape[1:], np.float32)
    r.reshape((H, DKP) + w.shape[1:])[:, :DK] = w.reshape((H, DK) + w.shape[1:])
    return r


def _vec_t(v, ko):
    """[ko*128] vector -> [128, ko] f32."""
    return np.ascontiguousarray(v.reshape(ko, P).T.astype(np.float32))


def _make_in_maps(inputs):
    inputs = {k: np.asarray(v) for k, v in inputs.items()}

    wq_pad = _col_pad_heads(inputs["wq_w"].astype(np.float32))
    wk_pad = _col_pad_heads(inputs["wk_w"].astype(np.float32))
    wv_pad = _col_pad_heads(inputs["wv_w"].astype(np.float32))
    wo_pad = _row_pad_heads(inputs["wo_w"].astype(np.float32))

    shared = {
        "vp_wt": _tile_w(inputs["vp_w"].astype(np.float32), KO_DV, KO_DM),
        "wq_t": _tile_w(wq_pad, KO_DM, KO_QP),
        "wk_t": _tile_w(wk_pad, KO_DM, KO_QP),
        "wv_r": _tile_w(wv_pad, KO_DM, DQP // 512),
        "wo_t": _tile_w(wo_pad, KO_QP, KO_DM),
        "f1_t": _tile_w(inputs["f1_w"].astype(np.float32), KO_DM, KO_DF),
        "f2_t": _tile_w(inputs["f2_w"].astype(np.float32), KO_DF, KO_DM),
        "vp_bt": _vec_t(inputs["vp_b"], KO_DM),
        "wqb_t": _vec_t(_col_pad_heads(inputs["wq_b"][None])[0] * SCALE, KO_QP),
        "wkb_t": _vec_t(_col_pad_heads(inputs["wk_b"][None])[0], KO_QP),
        "wvb": np.ascontiguousarray(
            _col_pad_heads(inputs["wv_b"][None]).astype(np.float32)),
        "wob_t": _vec_t(inputs["wo_b"], KO_DM),
        "f1b_t": _vec_t(inputs["f1_b"], KO_DF),
        "f2b_t": _vec_t(inputs["f2_b"], KO_DM),
        "ln1w_t": _vec_t(inputs["ln1_w"], KO_DM),
        "ln1b_t": _vec_t(inputs["ln1_b"], KO_DM),
        "ln2w_t": _vec_t(inputs["ln2_w"], KO_DM),
        "ln2b_t": _vec_t(inputs["ln2_b"], KO_DM),
    }

    text = inputs["text_embeddings"].astype(np.float32)
    vision = inputs["vision_features"].astype(np.float32)
    in_maps = []
    for b in range(B):
        te_b = np.ascontiguousarray(
            text[b].T.reshape(KO_DM, P, SQ).transpose(1, 0, 2).astype(bf16))
        vf_pad = np.zeros((DV, SVP), np.float32)
        vf_pad[:, :SV] = vision[b].T
        vf_b = np.ascontiguousarray(
            vf_pad.reshape(KO_DV, P, SVP).transpose(1, 0, 2).astype(bf16))
        in_maps.append({"te": te_b, "vf": vf_b, **shared})
    return in_maps


def kernel(**inputs):
    in_maps = _make_in_maps(inputs)

    if "nc" not in _NC_CACHE:
        _NC_CACHE["nc"] = _build_nc()
    nc = _NC_CACHE["nc"]

    res = run_bass_kernel_spmd(nc, in_maps, core_ids=list(range(B)))

    outs = []
    for b in range(B):
        r = res.results[b]["out"] + res.results[b]["x_out"]  # [128, 18, 2048]
        outs.append(r.transpose(1, 0, 2).reshape(DM, SQ).T)
    return np.stack(outs).astype(np.float32)


if __name__ == "__main__":
    import reference
    inp = {k: np.asarray(v) for k, v in reference.setup_inputs().items()}
    got = kernel(**inp)
    exp = np.asarray(reference.reference(**inp))
    err = float(np.linalg.norm(got - exp) / np.linalg.norm(exp))
    print("Relative error:", err)



# revision 25
# speedup vs baseline: 1.3476x; 1.3476x over previous
"""Trainium2 Bass kernel for nn_BridgeModule (vision->text cross-attention + FFN).

Data-parallel over batch (B=8, one batch element per NeuronCore), channel-major
dataflow (features on partitions, tokens on the free dim).

v2 design:
  - fp8e4(+DoubleRow) attention path: vision proj, K/V, Q, scores, ctx, O all
    run in fp8 (attention output is ~3.5% of |x|, so fp8 error is negligible
    in the final output); FFN stays bf16.
  - LayerNorms are folded into the consumer weights host-side:
      ln_w into wq/f1 rows; ln_b into wq_b/f1_b (exact).  Device computes only
      per-token mean/rstd rows and applies them during PSUM eviction
      (q = r*qraw + (-m*r)*colsum(wq') [+qb]); nt/nx are never materialized.
  - Residual is applied on device; single fp32 output.
  - fp8 tensors are scaled (weights x32, normalized ctx x16) to stay in
    e4m3's normal range; descales fold into PSUM-eviction scale slots.
  - All heads' softmax reciprocals are bounced through DRAM in one batch per
    1024-token block (partition-broadcast), not per head.
"""

import numpy as np
import ml_dtypes

import concourse.bass as bass
import concourse.tile as tile
import concourse.mybir as mybir
from concourse import bacc
from concourse.bass_utils import run_bass_kernel_spmd

# ---------------------------------------------------------------- constants
B, SV, SQ = 8, 257, 2048
DV, DM, H = 1024, 2304, 8
DK = DM // H            # 288
DKP = 384               # padded head dim (3 x 128)
DQP = H * DKP           # 3072
DF = 4 * DM             # 9216
SVP = 384               # padded vision tokens
EPS = 1e-5
P = 128
SCALE = 1.0 / float(np.sqrt(np.float32(DK)))

KO_DM = DM // P         # 18
KO_QP = DQP // P        # 24
KO_DV = DV // P         # 8
KO_DF = DF // P         # 72
HC = DKP // P           # 3 contraction chunks per head
ST = SVP // P           # 3 vision-token partition tiles
NB = 4                  # attention token blocks
NBS = SQ // NB          # 512
NT = SQ // 512          # matmul free-dim tiles of 512

WS = 32.0               # fp8 weight scale
CR = 16.0               # fp8 normalized-ctx scale

BF = mybir.dt.bfloat16
F32 = mybir.dt.float32
F8 = mybir.dt.float8e4
bf16 = ml_dtypes.bfloat16
f8e4 = ml_dtypes.float8_e4m3
DR = mybir.MatmulPerfMode.DoubleRow

AF = mybir.ActivationFunctionType
OP = mybir.AluOpType

_NC_CACHE = {}


def _dq(nc, i):
    """Alternate bulk DMAs between the two HW DGE queues (SP / ACT)."""
    return nc.sync if i % 2 == 0 else nc.scalar


def _pbcast(ap2d, p=P):
    """[1, ...] AP -> [p, ...] AP with partition stride 0 (for DMA broadcast)."""
    aplist = [list(x) for x in ap2d.ap]
    return bass.AP(tensor=ap2d.tensor, offset=ap2d.offset,
                   ap=[[0, p]] + aplist[1:])


def _build_nc(has_qb, has_wob):
    nc = bacc.Bacc(target_bir_lowering=False)
    with tile.TileContext(nc) as tc:
        _emit(nc, tc, has_qb, has_wob)
    nc.compile()
    return nc


def _emit(nc, tc, has_qb, has_wob):
    import os
    kph = int(os.environ.get("KPH", "9"))
    with tc.tile_pool(name="dram", bufs=1, space="DRAM") as dram:
        # ---------------- external I/O (SBUF-image layouts, host-prepped)
        def ein(name, shape, dtype):
            return dram.tile(list(shape), dtype, kind="ExternalInput",
                             name=name, uniquify=False)

        te = ein("te", [P, KO_DM, SQ], BF)
        te8 = ein("te8", [P, KO_DM, SQ], F8)
        vf8 = ein("vf8", [P, KO_DV, SVP], F8)
        vp8 = ein("vp8", [KO_DM, P, KO_DV, P], F8)
        wq8 = ein("wq8", [KO_QP, P, KO_DM, P], F8)
        wk8 = ein("wk8", [KO_QP, P, KO_DM, P], F8)
        wv8 = ein("wv8", [DQP // 512, P, KO_DM, 512], F8)
        wo8 = ein("wo8", [KO_DM, P, KO_QP, P], F8)
        f1t = ein("f1t", [KO_DF, P, KO_DM, P], BF)
        f2t = ein("f2t", [KO_DM, P, KO_DF, P], BF)
        vp_bt = ein("vp_bt", [P, KO_DM], F32)
        wkb_t = ein("wkb_t", [P, KO_QP], F32)
        qs_c = ein("qs_c", [P, KO_QP], F32)
        qb_c = ein("qb_c", [P, KO_QP], F32)
        wvb = ein("wvb", [1, DQP], F32)
        wob_t = ein("wob_t", [P, KO_DM], F32)
        f1b_t = ein("f1b_t", [P, KO_DF], F32)
        f1s_c = ein("f1s_c", [P, KO_DF], F32)
        f2b_t = ein("f2b_t", [P, KO_DM], F32)
        out = dram.tile([P, KO_DM, SQ], F32, kind="ExternalOutput",
                        name="out", uniquify=False)

        # DRAM scratch
        x_dram = dram.tile([P, KO_DM, SQ], BF, name="x_dram")
        rec_dram = dram.tile([1, NB * H * NBS], BF, name="rec_dram")

        with tc.tile_pool(name="consts", bufs=1) as consts, \
             tc.tile_pool(name="psum", bufs=4, space="PSUM") as psum, \
             tc.tile_pool(name="psums", bufs=4, space="PSUM") as psums:

            ones_bf = consts.tile([P, 1], BF)
            nc.vector.memset(ones_bf[:], 1.0)
            ones_f8 = consts.tile([P, 1], F8)
            nc.vector.memset(ones_f8[:], 1.0)

            def cload(src, shape):
                t = consts.tile(list(shape), F32, tag=f"c_{src.name}")
                nc.sync.dma_start(t[:], src[:])
                return t

            vp_b = cload(vp_bt, [P, KO_DM])
            wk_b = cload(wkb_t, [P, KO_QP])
            qs_v = cload(qs_c, [P, KO_QP])
            qb_v = cload(qb_c, [P, KO_QP]) if has_qb else None
            wo_b = cload(wob_t, [P, KO_DM]) if has_wob else None
            f1_b = cload(f1b_t, [P, KO_DF])
            f1s_v = cload(f1s_c, [P, KO_DF])
            f2_b = cload(f2b_t, [P, KO_DM])

            with tc.tile_pool(name="kvpool", bufs=1) as kvpool:
                kcm = kvpool.tile([P, KO_QP, SVP], F8)    # keys, channel-major
                v_tm = kvpool.tile([P, ST, DQP], F8)      # values, token-major
                q_dram = dram.tile([P, KO_QP, SQ], F8, name="q_dram")
                # te(+te8) streams on the sync queue from t=0 (LN1 stats
                # are the critical path for Q); vision weights ride the
                # scalar queue concurrently.
                with tc.tile_pool(name="l1pool", bufs=1) as l1pool, \
                     tc.tile_pool(name="te8p", bufs=1) as te8p:
                    te_sb = l1pool.tile([P, KO_DM, SQ], BF, tag="ln1_src")
                    te8_sb = te8p.tile([P, KO_DM, SQ], F8)
                    if kph >= 2:
                        for m in range(KO_DM):
                            nc.sync.dma_start(te_sb[:, m], te[:, m])
                    if kph >= 3:
                        for m in range(KO_DM):
                            nc.sync.dma_start(te8_sb[:, m], te8[:, m])
                    if kph >= 1:
                        _vision_kv(nc, tc, psum, vf8, vp8, wk8, wv8,
                                   vp_b, wk_b, wvb, kcm, v_tm)
                    if kph >= 2:
                        rq_d, mr_d = _ln_stats_sb(
                            nc, tc, psums, l1pool, ones_bf, te_sb, dram,
                            "ln1", 1.0 / WS)
                    with tc.tile_pool(name="l1bc", bufs=1) as l1bc:
                        if kph >= 3:
                            rq_b = l1bc.tile([P, SQ], BF, tag="rq_b")
                            nc.gpsimd.dma_start(rq_b[:], _pbcast(rq_d[:]))
                            mr_b = l1bc.tile([P, SQ], BF, tag="mr_b")
                            nc.gpsimd.dma_start(mr_b[:], _pbcast(mr_d[:]))
                            _q_proj(nc, tc, psum, te8_sb, wq8, rq_b, mr_b,
                                    qs_v, qb_v, q_dram)

                if kph >= 4:
                    _attention(nc, tc, psum, psums, ones_bf, ones_f8,
                               kcm, v_tm, q_dram, wo8, wo_b, te, x_dram,
                               rec_dram, dram)

            if kph >= 6:
                _ffn(nc, tc, psum, f1t, f2t, x_dram, _emit.r2_d,
                     _emit.mr2_d, f1s_v, f1_b, f2_b, out)


def _vision_kv(nc, tc, psum, vf8, vp8, wk8, wv8, vp_b, wk_b, wvb, kcm, v_tm):
    """pv = vp.T @ vf + vp_b (fp8); keys kcm = wk.T @ pv + wk_b (channel-major
    fp8); values v_tm = pv.T @ wv + wv_b (token-major fp8). All DoubleRow."""
    with tc.tile_pool(name="vision", bufs=1) as vision, \
         tc.tile_pool(name="vwork", bufs=3) as vwork:
        wv_bb = vision.tile([P, DQP], F32)
        nc.sync.dma_start(wv_bb[:], _pbcast(wvb[:]))
        vf_sb = vision.tile([P, KO_DV, SVP], F8)
        nc.sync.dma_start(vf_sb[:], vf8[:])
        pv = vision.tile([P, KO_DM, SVP], F8)
        for m in range(KO_DM):
            w_sl = vwork.tile([P, KO_DV, P], F8, tag="vp_sl")
            _dq(nc, m).dma_start(w_sl[:], vp8[m])
            ps = psum.tile([P, 512], F32, tag="ps_a")
            for k2 in range(KO_DV // 2):
                nc.tensor.matmul(ps[:, :SVP], w_sl[:, 2 * k2:2 * k2 + 2],
                                 vf_sb[:, 2 * k2:2 * k2 + 2],
                                 start=(k2 == 0), stop=(k2 == KO_DV // 2 - 1),
                                 perf_mode=DR)
            nc.scalar.activation(pv[:, m], ps[:, :SVP], AF.Identity,
                                 bias=vp_b[:, m:m + 1], scale=1.0 / WS)

        for m in range(KO_QP):
            w_sl = vwork.tile([P, KO_DM, P], F8, tag="wk_sl")
            _dq(nc, m).dma_start(w_sl[:], wk8[m])
            ps = psum.tile([P, 512], F32, tag="ps_a")
            for k2 in range(KO_DM // 2):
                nc.tensor.matmul(ps[:, :SVP], w_sl[:, 2 * k2:2 * k2 + 2],
                                 pv[:, 2 * k2:2 * k2 + 2],
                                 start=(k2 == 0), stop=(k2 == KO_DM // 2 - 1),
                                 perf_mode=DR)
            nc.scalar.activation(kcm[:, m], ps[:, :SVP], AF.Identity,
                                 bias=wk_b[:, m:m + 1], scale=1.0 / WS)

        for n in range(DQP // 512):
            w_sl = vwork.tile([P, KO_DM, 512], F8, tag="wv_sl", bufs=2)
            _dq(nc, n).dma_start(w_sl[:], wv8[n])
            for st in range(ST):
                ps = psum.tile([P, 512], F32, tag="ps_a")
                ssl = slice(st * P, (st + 1) * P)
                for k2 in range(KO_DM // 2):
                    nc.tensor.matmul(ps[:], pv[:, 2 * k2:2 * k2 + 2, ssl],
                                     w_sl[:, 2 * k2:2 * k2 + 2],
                                     start=(k2 == 0),
                                     stop=(k2 == KO_DM // 2 - 1),
                                     perf_mode=DR)
                nc.vector.scalar_tensor_tensor(
                    v_tm[:, st, n * 512:(n + 1) * 512], ps[:], 1.0 / WS,
                    wv_bb[:, n * 512:(n + 1) * 512], OP.mult, OP.add)


def _ln_stats(nc, tc, psums, pool, ones_bf, src_dram, dram, nm, rscale):
    """Per-token mean/rstd of a [P, KO_DM, SQ] bf16 DRAM tensor (reduction over
    channels=partitions via ones-matmuls).  Returns DRAM rows (rstd*rscale,
    -mean*rstd) ready for partition-broadcast."""
    src_sb = pool.tile([P, KO_DM, SQ], BF, tag=nm + "_src")
    for m in range(KO_DM):
        _dq(nc, m).dma_start(src_sb[:, m], src_dram[:, m])
    return _ln_stats_sb(nc, tc, psums, pool, ones_bf, src_sb, dram, nm, rscale)


def _ln_stats_sb(nc, tc, psums, pool, ones_bf, src_sb, dram, nm, rscale):
    with tc.tile_pool(name=nm + "w", bufs=3) as work:
        sums = pool.tile([1, SQ], F32, tag=nm + "_sums")
        sumsq = pool.tile([1, SQ], F32, tag=nm + "_sumsq")
        for n in range(NT):
            nsl = slice(n * 512, (n + 1) * 512)
            ps2 = psums.tile([33, 512], F32, tag="ps_st")
            for m in range(KO_DM):
                nc.tensor.matmul(ps2[0:1], ones_bf[:], src_sb[:, m, nsl],
                                 start=(m == 0), stop=(m == KO_DM - 1))
            for m in range(KO_DM):
                sq = work.tile([P, 512], BF, tag=nm + "_sq")
                nc.vector.tensor_mul(sq[:], src_sb[:, m, nsl],
                                     src_sb[:, m, nsl])
                nc.tensor.matmul(ps2[32:33], ones_bf[:], sq[:],
                                 start=(m == 0), stop=(m == KO_DM - 1))
            nc.vector.tensor_copy(sums[:, nsl], ps2[0:1])
            nc.vector.tensor_copy(sumsq[:, nsl], ps2[32:33])
        return _ln_finalize(nc, pool, sums, sumsq, dram, nm, rscale)


def _ln_finalize(nc, pool, sums, sumsq, dram, nm, rscale):
    """-> DRAM rows (rstd*rscale [1,SQ], -mean*rstd [1,SQ])."""
    tmp = pool.tile([1, SQ], F32, tag=nm + "_fin_tmp")
    nc.vector.tensor_scalar_mul(sums[:], sums[:], 1.0 / DM)      # mean
    nc.vector.tensor_scalar_mul(sumsq[:], sumsq[:], 1.0 / DM)
    nc.vector.scalar_tensor_tensor(tmp[:], sums[:], 1.0, sums[:],
                                   OP.mult, OP.mult)             # mean^2
    nc.vector.tensor_sub(sumsq[:], sumsq[:], tmp[:])             # var
    eps_t = pool.tile([1, 1], F32, tag=nm + "_eps")
    nc.vector.memset(eps_t[:], EPS)
    nc.scalar.activation(tmp[:], sumsq[:], AF.Sqrt, bias=eps_t[:])  # std
    nc.vector.reciprocal(sumsq[:], tmp[:])                       # rstd
    nc.vector.scalar_tensor_tensor(tmp[:], sums[:], -1.0, sumsq[:],
                                   OP.mult, OP.mult)             # -mean*rstd
    if rscale != 1.0:
        nc.vector.tensor_scalar_mul(sumsq[:], sumsq[:], rscale)
    # row bounces ride the SWDGE queue so they never block the HWDGE
    # weight/activation FIFOs
    r_d = dram.tile([1, SQ], F32, name=nm + "_r_dram")
    nc.gpsimd.dma_start(r_d[:], sumsq[:])
    mr_d = dram.tile([1, SQ], F32, name=nm + "_mr_dram")
    nc.gpsimd.dma_start(mr_d[:], tmp[:])
    return r_d, mr_d


def _q_proj(nc, tc, psum, te8_sb, wq8, rq_b, mr_b, qs_v, qb_v, q_dram):
    """q = rstd*(wq'.T @ te) + (-mean*rstd)*colsum(wq') [+qb] -> fp8 DRAM.
    (ln1 folded host-side; q NOT pre-scaled by 1/sqrt(dk).)"""
    with tc.tile_pool(name="qwork", bufs=3) as qwork:
        for m in range(KO_QP):
            w_sl = qwork.tile([P, KO_DM, P], F8, tag="wq_sl")
            _dq(nc, m).dma_start(w_sl[:], wq8[m])
            for n in range(NT):
                nsl = slice(n * 512, (n + 1) * 512)
                ps = psum.tile([P, 512], F32, tag="ps_a")
                for k2 in range(KO_DM // 2):
                    nc.tensor.matmul(ps[:], w_sl[:, 2 * k2:2 * k2 + 2],
                                     te8_sb[:, 2 * k2:2 * k2 + 2, nsl],
                                     start=(k2 == 0),
                                     stop=(k2 == KO_DM // 2 - 1),
                                     perf_mode=DR)
                tmp = qwork.tile([P, 512], F32, tag="q_tmp")
                nc.vector.tensor_mul(tmp[:], ps[:], rq_b[:, nsl])
                q_sl = qwork.tile([P, 512], F8, tag="q_sl")
                if qb_v is None:
                    nc.vector.scalar_tensor_tensor(
                        q_sl[:], mr_b[:, nsl], qs_v[:, m:m + 1],
                        tmp[:], OP.mult, OP.add)
                else:
                    nc.vector.scalar_tensor_tensor(
                        tmp[:], mr_b[:, nsl], qs_v[:, m:m + 1],
                        tmp[:], OP.mult, OP.add)
                    nc.vector.tensor_scalar_add(q_sl[:], tmp[:],
                                                qb_v[:, m:m + 1])
                _dq(nc, m + n).dma_start(q_dram[:, m, nsl], q_sl[:])


def _attention(nc, tc, psum, psums, ones_bf, ones_f8, kcm, v_tm, q_dram, wo8,
               wo_b, te, x_dram, rec_dram, dram):
    """Blocked cross-attention + O projection + residual + LN2 stats.
    Scores for both blocks first (hides the reciprocal DRAM bounce), then
    ctx/O per block.  x -> x_dram (bf16)."""
    with tc.tile_pool(name="attn", bufs=2) as attn, \
         tc.tile_pool(name="awork", bufs=2) as awork:
        sums2 = awork.tile([1, SQ], F32, tag="x2s", bufs=1, name="x2sums")
        sumsq2 = awork.tile([1, SQ], F32, tag="x2q", bufs=1, name="x2sumsq")

        def scores(nb):
            q_blk = attn.tile([P, KO_QP, NBS], F8, tag="q_blk",
                              name=f"q_blk{nb}")
            _dq(nc, nb).dma_start(q_blk[:], q_dram[:, :,
                                                   nb * NBS:(nb + 1) * NBS])
            expT = attn.tile([P, H * ST, NBS], F8, tag="expT",
                             name=f"expT{nb}")
            rec = awork.tile([1, H * NBS], BF, tag="rec", name=f"rec{nb}")
            for h in range(H):
                nc.vector.memset(expT[:, HC * h + HC - 1], 0.0)
                ps_sum = psums.tile([33, NBS], F32, tag="ps_st",
                                    name=f"ps_sum{nb}_{h}")
                for st in range(ST):
                    ps_s = psum.tile([P, NBS], F32, tag="ps_a",
                                     name=f"ps_s{nb}_{h}_{st}")
                    ssl = slice(st * P, (st + 1) * P)
                    nc.tensor.matmul(ps_s[:], kcm[:, HC * h:HC * h + 2, ssl],
                                     q_blk[:, HC * h:HC * h + 2],
                                     start=True, stop=False, perf_mode=DR)
                    nc.tensor.matmul(ps_s[:], kcm[:, HC * h + 2, ssl],
                                     q_blk[:, HC * h + 2],
                                     start=False, stop=True)
                    if st < ST - 1:
                        nc.scalar.activation(expT[:, HC * h + st], ps_s[:],
                                             AF.Exp, scale=SCALE)
                    else:
                        nc.scalar.activation(expT[0:1, HC * h + st],
                                             ps_s[0:1], AF.Exp, scale=SCALE)
                    nc.tensor.matmul(ps_sum[0:1], ones_f8[:],
                                     expT[:, HC * h + st],
                                     start=(st == 0), stop=(st == ST - 1))
                nc.vector.tensor_scalar_mul(ps_sum[0:1], ps_sum[0:1],
                                            1.0 / CR)
                with nc.allow_low_precision(
                        reason="softmax recip in bf16 is plenty"):
                    nc.vector.reciprocal(rec[:, h * NBS:(h + 1) * NBS],
                                         ps_sum[0:1])
            roff = nb * H * NBS
            nc.gpsimd.dma_start(rec_dram[:, roff:roff + H * NBS], rec[:])
            rec_b = attn.tile([P, H * NBS], BF, tag="rec_b",
                              name=f"rec_b{nb}")
            nc.gpsimd.dma_start(rec_b[:],
                                _pbcast(rec_dram[:, roff:roff + H * NBS]))
            return expT, rec_b

        def ctx_o(nb, expT, rec_b):
            bsl = slice(nb * NBS, (nb + 1) * NBS)
            ctx8 = attn.tile([P, KO_QP, NBS], F8, tag="ctx8",
                             name=f"ctx8_{nb}")
            for h in range(H):
                for d3 in range(HC):
                    dsl = slice((HC * h + d3) * P, (HC * h + d3 + 1) * P)
                    ps_c = psum.tile([P, NBS], F32, tag="ps_a",
                                     name=f"ps_c{nb}_{h}_{d3}")
                    nc.tensor.matmul(ps_c[:], v_tm[:, 0:2, dsl],
                                     expT[:, HC * h:HC * h + 2],
                                     start=True, stop=False, perf_mode=DR)
                    nc.tensor.matmul(ps_c[:], v_tm[:, 2, dsl],
                                     expT[:, HC * h + 2],
                                     start=False, stop=True)
                    nc.vector.tensor_mul(ctx8[:, HC * h + d3], ps_c[:],
                                         rec_b[:, h * NBS:(h + 1) * NBS])

            # O projection + residual -> x_dram (bf16); LN2 stats inline.
            ps_st = psums.tile([33, NBS], F32, tag="ps_st",
                               name=f"ps_st{nb}")
            for m in range(KO_DM):
                w_sl = awork.tile([P, KO_QP, P], F8, tag="wo_sl", bufs=3)
                _dq(nc, m).dma_start(w_sl[:], wo8[m])
                te_sl = awork.tile([P, NBS], BF, tag="te_res")
                _dq(nc, m + 1).dma_start(te_sl[:], te[:, m, bsl])
                x_t = awork.tile([P, NBS], BF, tag="x_t")
                sq_t = awork.tile([P, NBS], BF, tag="sq_t")
                ps = psum.tile([P, NBS], F32, tag="ps_a",
                               name=f"ps_o{nb}_{m}")
                for k2 in range(KO_QP // 2):
                    nc.tensor.matmul(ps[:], w_sl[:, 2 * k2:2 * k2 + 2],
                                     ctx8[:, 2 * k2:2 * k2 + 2],
                                     start=(k2 == 0),
                                     stop=(k2 == KO_QP // 2 - 1),
                                     perf_mode=DR)
                if wo_b is None:
                    nc.vector.scalar_tensor_tensor(
                        x_t[:], ps[:], 1.0 / (WS * CR), te_sl[:],
                        OP.mult, OP.add)
                else:
                    tmp = awork.tile([P, NBS], F32, tag="xo_tmp")
                    nc.vector.scalar_tensor_tensor(
                        tmp[:], ps[:], 1.0 / (WS * CR), wo_b[:, m:m + 1],
                        OP.mult, OP.add)
                    nc.vector.tensor_add(x_t[:], tmp[:], te_sl[:])
                nc.tensor.matmul(ps_st[0:1], ones_bf[:], x_t[:],
                                 start=(m == 0), stop=(m == KO_DM - 1))
                nc.vector.tensor_mul(sq_t[:], x_t[:], x_t[:])
                nc.tensor.matmul(ps_st[32:33], ones_bf[:], sq_t[:],
                                 start=(m == 0), stop=(m == KO_DM - 1))
                _dq(nc, m).dma_start(x_dram[:, m, bsl], x_t[:])
            nc.vector.tensor_copy(sums2[:, bsl], ps_st[0:1])
            nc.vector.tensor_copy(sumsq2[:, bsl], ps_st[32:33])

        # scores run one block ahead of ctx/O to hide the reciprocal bounce
        pend = [scores(0)]
        for nb in range(NB):
            if nb + 1 < NB:
                pend.append(scores(nb + 1))
            ctx_o(nb, *pend[nb])
        _emit.r2_d, _emit.mr2_d = _ln_finalize(
            nc, awork, sums2, sumsq2, dram, "ln2", 1.0)


def _ffn(nc, tc, psum, f1t, f2t, x_dram, r2_d, mr2_d, f1s_v, f1_b, f2_b,
         out):
    """Fused FFN over 512-token phases; h never leaves SBUF.
    FFN1(n): h = gelu(r2*(f1'.T @ x) + (-m2*r2)*colsum(f1') + f1_b')
    FFN2(n): out = f2.T @ h + f2_b + x   (residual on device)
    f1 streams on the sync queue, f2 on the scalar queue; both are re-read
    per phase (~170 MB each over the whole FFN span -- well under the HBM
    budget and fully overlapped)."""
    with tc.tile_pool(name="hpool", bufs=1) as hpool, \
         tc.tile_pool(name="fwork", bufs=2) as fwork:
        for n in range(NT):
            nsl = slice(n * 512, (n + 1) * 512)
            x_blk = fwork.tile([P, KO_DM, 512], BF, tag="x_blk",
                               name=f"x_blk{n}")
            nc.gpsimd.dma_start(x_blk[:], x_dram[:, :, nsl])
            r2s = fwork.tile([P, 512], F32, tag="r2s", name=f"r2s{n}")
            nc.gpsimd.dma_start(r2s[:], _pbcast(r2_d[:, nsl]))
            mr2s = fwork.tile([P, 512], F32, tag="mr2s", name=f"mr2s{n}")
            nc.gpsimd.dma_start(mr2s[:], _pbcast(mr2_d[:, nsl]))
            h_sb = hpool.tile([P, KO_DF, 512], BF, tag="h_sb",
                              name=f"h_sb{n}")
            for m in range(KO_DF):
                w_sl = fwork.tile([P, KO_DM, P], BF, tag="f1_sl", bufs=3)
                nc.sync.dma_start(w_sl[:], f1t[m])
                ps = psum.tile([P, 512], F32, tag="ps_a",
                               name=f"ps_f1_{n}_{m}")
                for k in range(KO_DM):
                    nc.tensor.matmul(ps[:], w_sl[:, k], x_blk[:, k],
                                     start=(k == 0), stop=(k == KO_DM - 1))
                tmp = fwork.tile([P, 512], F32, tag="h_tmp", bufs=3)
                nc.vector.tensor_mul(tmp[:], ps[:], r2s[:])
                nc.vector.scalar_tensor_tensor(tmp[:], mr2s[:],
                                               f1s_v[:, m:m + 1], tmp[:],
                                               OP.mult, OP.add)
                nc.scalar.activation(h_sb[:, m], tmp[:], AF.Gelu,
                                     bias=f1_b[:, m:m + 1])
            for m in range(KO_DM):
                w2_sl = fwork.tile([P, KO_DF, P], BF, tag="f2_sl")
                nc.scalar.dma_start(w2_sl[:], f2t[m])
                ps = psum.tile([P, 512], F32, tag="ps_a",
                               name=f"ps_f2_{n}_{m}")
                for k in range(KO_DF):
                    nc.tensor.matmul(ps[:], w2_sl[:, k], h_sb[:, k],
                                     start=(k == 0), stop=(k == KO_DF - 1))
                o_sl = fwork.tile([P, 512], F32, tag="o_sl")
                nc.vector.scalar_tensor_tensor(
                    o_sl[:], ps[:], f2_b[:, m:m + 1], x_blk[:, m],
                    OP.add, OP.add)
                _dq(nc, m).dma_start(out[:, m, nsl], o_sl[:])


# ------------------------------------------------------------- host wrappers

def _tile_w(w, ko, mo, dtype):
    """[K, M] weight -> [mo, 128, ko, mi] SBUF-image tiles."""
    K, M = w.shape
    mi = M // mo
    r = w.reshape(ko, P, mo, mi).transpose(2, 1, 0, 3)
    return np.ascontiguousarray(r.astype(dtype))


def _col_pad_heads(w):
    """[*, 2304] -> [*, 3072] zero-padding each head's 288 cols to 384."""
    r = np.zeros(w.shape[:-1] + (DQP,), np.float32)
    r.reshape(w.shape[:-1] + (H, DKP))[..., :DK] = \
        w.reshape(w.shape[:-1] + (H, DK))
    return r


def _row_pad_heads(w):
    """[2304, *] -> [3072, *] zero-padding each head's 288 rows to 384."""
    r = np.zeros((DQP,) + w.shape[1:], np.float32)
    r.reshape((H, DKP) + w.shape[1:])[:, :DK] = w.reshape((H, DK) + w.shape[1:])
    return r


def _vec_t(v, ko):
    """[ko*128] vector -> [128, ko] f32."""
    return np.ascontiguousarray(v.reshape(ko, P).T.astype(np.float32))


def _make_in_maps(inputs):
    inputs = {k: np.asarray(v, np.float32) for k, v in inputs.items()}

    ln1w = inputs["ln1_w"][:, None]
    ln2w = inputs["ln2_w"][:, None]
    wq_f = _col_pad_heads(inputs["wq_w"] * ln1w)        # ln1_w folded
    wk_pad = _col_pad_heads(inputs["wk_w"])
    wv_pad = _col_pad_heads(inputs["wv_w"])
    wo_pad = _row_pad_heads(inputs["wo_w"])
    f1_f = inputs["f1_w"] * ln2w                        # ln2_w folded

    qb = _col_pad_heads((inputs["wq_b"]
                         + inputs["ln1_b"] @ inputs["wq_w"])[None])[0]
    f1b = inputs["f1_b"] + inputs["ln2_b"] @ inputs["f1_w"]
    has_qb = bool(np.any(qb))
    has_wob = bool(np.any(inputs["wo_b"]))

    shared = {
        "vp8": _tile_w(inputs["vp_w"] * WS, KO_DV, KO_DM, f8e4),
        "wq8": _tile_w(wq_f * WS, KO_DM, KO_QP, f8e4),
        "wk8": _tile_w(wk_pad * WS, KO_DM, KO_QP, f8e4),
        "wv8": _tile_w(wv_pad * WS, KO_DM, DQP // 512, f8e4),
        "wo8": _tile_w(wo_pad * WS, KO_QP, KO_DM, f8e4),
        "f1t": _tile_w(f1_f, KO_DM, KO_DF, bf16),
        "f2t": _tile_w(inputs["f2_w"], KO_DF, KO_DM, bf16),
        "vp_bt": _vec_t(inputs["vp_b"], KO_DM),
        "wkb_t": _vec_t(_col_pad_heads(inputs["wk_b"][None])[0], KO_QP),
        "qs_c": _vec_t(wq_f.sum(axis=0), KO_QP),
        "qb_c": _vec_t(qb, KO_QP),
        "wvb": np.ascontiguousarray(
            _col_pad_heads(inputs["wv_b"][None]).astype(np.float32)),
        "wob_t": _vec_t(inputs["wo_b"], KO_DM),
        "f1b_t": _vec_t(f1b, KO_DF),
        "f1s_c": _vec_t(f1_f.sum(axis=0), KO_DF),
        "f2b_t": _vec_t(inputs["f2_b"], KO_DM),
    }

    text = inputs["text_embeddings"]
    vision = inputs["vision_features"]
    in_maps = []
    for b in range(B):
        te_cm = np.ascontiguousarray(
            text[b].T.reshape(KO_DM, P, SQ).transpose(1, 0, 2))
        vf_pad = np.zeros((DV, SVP), np.float32)
        vf_pad[:, :SV] = vision[b].T
        vf_b = np.ascontiguousarray(
            vf_pad.reshape(KO_DV, P, SVP).transpose(1, 0, 2).astype(f8e4))
        in_maps.append({"te": te_cm.astype(bf16), "te8": te_cm.astype(f8e4),
                        "vf8": vf_b, **shared})
    return in_maps, has_qb, has_wob


def kernel(**inputs):
    in_maps, has_qb, has_wob = _make_in_maps(inputs)

    key = ("nc", has_qb, has_wob)
    if key not in _NC_CACHE:
        _NC_CACHE[key] = _build_nc(has_qb, has_wob)
        _NC_CACHE["nc"] = _NC_CACHE[key]
    nc = _NC_CACHE[key]

    res = run_bass_kernel_spmd(nc, in_maps, core_ids=list(range(B)))

    outs = []
    for b in range(B):
        r = res.results[b]["out"]                       # [128, 18, 2048]
        outs.append(r.transpose(1, 0, 2).reshape(DM, SQ).T)
    return np.stack(outs).astype(np.float32)


if __name__ == "__main__":
    import reference
    inp = {k: np.asarray(v) for k, v in reference.setup_inputs().items()}
    got = kernel(**inp)
    exp = np.asarray(reference.reference(**inp))
    err = float(np.linalg.norm(got - exp) / np.linalg.norm(exp))
    print("Relative error:", err)
